# revision 26
# baseline (speedup 1.0000x reference)
"""DeepseekV3 decode layer (MLA attention + SwiGLU MLP) on 8 Trainium2 NeuronCores.

v2: bf16 weights/cache (host-cast), host-pretransposed cache in BOTH layouts
([d,s] for scores, [s,l] for o_lat) to eliminate on-device cache transposes,
host-pretransposed W_UK, latent token folded into attention via K=1 matmuls.

Sharding:
  - A-projections (w_q_a, w_kv_a): row-sharded over HID, partials AllReduced.
  - q_b / kv_b up-projections: column-sharded over heads (4 heads/core).
  - Attention: data-parallel over batch (4 batches/core, full 32 heads);
    q and o_lat resharded via AllToAll (bf16 payloads).
  - w_o: row-sharded (by local heads' V blocks), partials AllReduced -> h.
  - MLP: gate/up column-sharded, down row-sharded; final partial-sum
    reduction on the host (h + sum_i mlp_i).
Softmax uses no max-subtraction (scores are O(5) at this scale).
"""
import numpy as np
import ml_dtypes
from contextlib import ExitStack

import concourse.bass as bass
import concourse.mybir as mybir
import concourse.tile as tile
from concourse import bacc
from concourse.bass_utils import run_bass_kernel_spmd
from concourse.masks import make_identity

F32 = mybir.dt.float32
BF16 = mybir.dt.bfloat16
AF = mybir.ActivationFunctionType
AXX = mybir.AxisListType.X

NC_N = 8
B, S, HID = 32, 4096, 4096
H, NOPE, ROPE, V, KVL, QL, INTER = 32, 128, 64, 128, 512, 1536, 11008
HEAD = NOPE + ROPE
D = KVL + ROPE            # 576
SCALE = HEAD ** -0.5
EPS = 1e-6
B_LOC, H_LOC = B // NC_N, H // NC_N          # 4, 4
HID_LOC = HID // NC_N                        # 512
I_LOC = INTER // NC_N                        # 1376
NST = 8                                      # 8 s-tiles of 512; latent separate

_CACHE = {}


def _build(reps=1, fake_coll=()):
    fake_coll = set(fake_coll) if not isinstance(fake_coll, bool) else ({'c1','c2','c3','c4'} if fake_coll else set())
    nc = bacc.Bacc("TRN2", target_bir_lowering=False, debug=False, num_devices=NC_N)

    # ---------------- I/O ----------------
    hidden = nc.dram_tensor("hidden", [B, HID], F32, kind="ExternalInput").ap()
    hidden_loc = nc.dram_tensor("hidden_loc", [B, HID_LOC], F32, kind="ExternalInput").ap()
    cos_i = nc.dram_tensor("cos", [B, ROPE // 2], F32, kind="ExternalInput").ap()
    sin_i = nc.dram_tensor("sin", [B, ROPE // 2], F32, kind="ExternalInput").ap()
    # host-packed cache tiles: one contiguous DMA per (batch, s-tile)
    # cacheTp[b, st, p, 512*dd + s] = cacheT_zeropad[b, 128*dd + p, 512*st + s]
    cacheTp = nc.dram_tensor("cacheTp", [B_LOC, NST, 128, 2560], BF16, kind="ExternalInput").ap()
    # cachekvp[b, st, p, 512*cc + l] = kv_cache[b, 512*st + 128*cc + p, l]
    cachekvp = nc.dram_tensor("cachekvp", [B_LOC, NST, 128, 2048], BF16, kind="ExternalInput").ap()
    wqa = nc.dram_tensor("wqa", [HID_LOC, QL], BF16, kind="ExternalInput").ap()
    wkva = nc.dram_tensor("wkva", [HID_LOC, D], BF16, kind="ExternalInput").ap()
    wqb = nc.dram_tensor("wqb", [QL, H_LOC * HEAD], BF16, kind="ExternalInput").ap()
    ukt_i = nc.dram_tensor("ukt", [NOPE, H_LOC * KVL], BF16, kind="ExternalInput").ap()
    wuv_i = nc.dram_tensor("wuv", [KVL, H_LOC * V], BF16, kind="ExternalInput").ap()
    wo = nc.dram_tensor("wo", [H_LOC * V, HID], BF16, kind="ExternalInput").ap()
    wgu = nc.dram_tensor("wgu", [HID, 2 * I_LOC], BF16, kind="ExternalInput").ap()
    wdn = nc.dram_tensor("wdn", [I_LOC, HID], BF16, kind="ExternalInput").ap()
    ln_in_loc = nc.dram_tensor("ln_in_loc", [32, HID_LOC], F32, kind="ExternalInput").ap()
    ln_q = nc.dram_tensor("ln_q", [32, QL], F32, kind="ExternalInput").ap()
    ln_kv = nc.dram_tensor("ln_kv", [32, KVL], F32, kind="ExternalInput").ap()
    ln_post = nc.dram_tensor("ln_post", [32, HID], F32, kind="ExternalInput").ap()
    bsel = nc.dram_tensor("bsel", [B, B_LOC], BF16, kind="ExternalInput").ap()

    h_out = nc.dram_tensor("h", [B, HID], F32, kind="ExternalOutput").ap()
    mlp_out = nc.dram_tensor("mlp", [B, HID], F32, kind="ExternalOutput").ap()

    with ExitStack() as ctx:
        tc = ctx.enter_context(tile.TileContext(nc))
        # ---------------- pools ----------------
        singles = ctx.enter_context(tc.tile_pool(name="singles", bufs=1))
        rows = ctx.enter_context(tc.tile_pool(name="rows", bufs=2))      # [32,<=4096] f32
        stats = ctx.enter_context(tc.tile_pool(name="stats", bufs=8))
        tmp = ctx.enter_context(tc.tile_pool(name="tmp", bufs=2))
        trsb = ctx.enter_context(tc.tile_pool(name="trsb", bufs=1))      # SBUF landings of transposes
        wsm = ctx.enter_context(tc.tile_pool(name="wsm", bufs=1))        # small streamed weights
        wbig = ctx.enter_context(tc.tile_pool(name="wbig", bufs=4))      # [128,<=2048] bf16 streamed
        wgup = ctx.enter_context(tc.tile_pool(name="wgup", bufs=3))      # [128,2752] gate+up tiles
        wdp = ctx.enter_context(tc.tile_pool(name="wdp", bufs=2))        # [128,4096] down/wo tiles
        ctp = ctx.enter_context(tc.tile_pool(name="ctp", bufs=3))        # cacheT groups [128,2560]
        kvp = ctx.enter_context(tc.tile_pool(name="kvp", bufs=3))        # cachekv groups [128,2048]
        apool = ctx.enter_context(tc.tile_pool(name="attn", bufs=2))     # attn per s-tile
        atp = ctx.enter_context(tc.tile_pool(name="attnT", bufs=3))
        mpool = ctx.enter_context(tc.tile_pool(name="misc", bufs=2))
        qtbp = ctx.enter_context(tc.tile_pool(name="qtb", bufs=4))
        dram = ctx.enter_context(tc.tile_pool(name="dram", bufs=1, space="DRAM"))

        ps_tr = ctx.enter_context(tc.tile_pool(name="ps_tr", bufs=2, space="PSUM"))
        ps_sc = ctx.enter_context(tc.tile_pool(name="ps_sc", bufs=1, space="PSUM"))
        ps_ct4 = ctx.enter_context(tc.tile_pool(name="ps_ct4", bufs=4, space="PSUM"))
        ps_vps = ctx.enter_context(tc.tile_pool(name="ps_vps", bufs=1, space="PSUM"))

        def rms_rstd(x_ap, dsz):
            """rstd [32,1] = 1/sqrt(mean(x^2)+eps) via one-pass bn_stats."""
            nsub = max(1, dsz // 512)
            stv = stats.tile([32, nsub, 6], F32, tag="bst", bufs=2, name="bst")
            xv = x_ap.rearrange("p (n f) -> p n f", n=nsub)
            for ss in range(nsub):
                nc.vector.bn_stats(out=stv[:, ss, :], in_=xv[:, ss, :])
            mv = stats.tile([32, 2], F32, tag="bmv", bufs=2, name="bmv")
            nc.vector.bn_aggr(out=mv[:, :], in_=stv[:, :, :])
            m2 = stats.tile([32, 1], F32, tag="st", name="m2")
            nc.vector.tensor_mul(m2[:, :], mv[:, 0:1], mv[:, 0:1])
            msq = stats.tile([32, 1], F32, tag="st", name="msq")
            nc.vector.tensor_add(msq[:, :], mv[:, 1:2], m2[:, :])
            rt = stats.tile([32, 1], F32, tag="st", name="rt")
            nc.scalar.activation(out=rt[:, :], in_=msq[:, :], func=AF.Sqrt,
                                 bias=eps_t[:, :], scale=1.0)
            rec = stats.tile([32, 1], F32, tag="st", name="rec")
            nc.vector.reciprocal(rec[:, :], rt[:, :])
            return rec

        def rope(out1_ap, out2_ap, x_even, x_odd):
            """out1 = even*cos - odd*sin ; out2 = odd*cos + even*sin."""
            t1 = tmp.tile([32, 32], F32, tag="ropet", bufs=4, name="t1")
            t2 = tmp.tile([32, 32], F32, tag="ropet", bufs=4, name="t2")
            nc.vector.tensor_mul(t1[:, :], x_even, cos_t[:, :])
            nc.vector.tensor_mul(t2[:, :], x_odd, sin_t[:, :])
            nc.vector.tensor_sub(out1_ap, t1[:, :], t2[:, :])
            t3 = tmp.tile([32, 32], F32, tag="ropet", bufs=4, name="t3")
            t4 = tmp.tile([32, 32], F32, tag="ropet", bufs=4, name="t4")
            nc.vector.tensor_mul(t3[:, :], x_odd, cos_t[:, :])
            nc.vector.tensor_mul(t4[:, :], x_even, sin_t[:, :])
            nc.vector.tensor_add(out2_ap, t3[:, :], t4[:, :])

        def bcast32(src_ap, dsz, name):
            t = rows.tile([32, dsz], F32, tag="rows", name=name)
            nc.sync.dma_start(out=t[:, :], in_=src_ap)
            return t

        # ---------------- constants / small inputs ----------------
        ident = singles.tile([128, 128], BF16)
        make_identity(nc, ident[:, :])
        id32 = ident[0:32, 0:32]
        sum_sel = singles.tile([128, 32], BF16)
        for a in range(4):
            nc.sync.dma_start(out=sum_sel[32 * a:32 * a + 32, :], in_=ident[0:32, 0:32])

        hidl_t = singles.tile([B, HID_LOC], F32)
        nc.sync.dma_start(out=hidl_t[:, :], in_=hidden_loc[:, :])
        cos_t = singles.tile([B, ROPE // 2], F32)
        nc.sync.dma_start(out=cos_t[:, :], in_=cos_i[:, :])
        sin_t = singles.tile([B, ROPE // 2], F32)
        nc.sync.dma_start(out=sin_t[:, :], in_=sin_i[:, :])
        bsel_t = singles.tile([B, B_LOC], BF16)
        nc.sync.dma_start(out=bsel_t[:, :], in_=bsel[:, :])
        eps_t = singles.tile([32, 1], F32)
        nc.vector.memset(eps_t[:, :], EPS)

        # collective bounce buffers
        c1i = dram.tile([B, QL + D], BF16, tag="c1i")
        c1o = dram.tile([B, QL + D], BF16, tag="c1o")
        c2i = dram.tile([B, H_LOC * D], BF16, tag="c2i")
        c2o = dram.tile([B, H_LOC * D], BF16, tag="c2o")
        c3i = dram.tile([B, H_LOC * KVL], BF16, tag="c3i")
        c3o = dram.tile([B, H_LOC * KVL], BF16, tag="c3o")
        c4i = dram.tile([B, HID], BF16, tag="c4i")
        c4o = dram.tile([B, HID], BF16, tag="c4o")
        rg = [list(range(NC_N))]

        for _rep in range(reps):
            # ============ Phase B: local x_ln slice ============
            hid_b = rows.tile([B, HID], F32, tag="rows", name="hid_b")
            nc.sync.dma_start(out=hid_b[:, :], in_=hidden[:, :])
            rstd_h = rms_rstd(hid_b[:, :], HID)
            lninl_b = bcast32(ln_in_loc[:, :], HID_LOC, "lninl_b")
            xlnl = singles.tile([B, HID_LOC], BF16)
            nc.vector.scalar_tensor_tensor(xlnl[:, :], hidl_t[:, :], rstd_h[:, :],
                                           lninl_b[:, :], mybir.AluOpType.mult, mybir.AluOpType.mult)

            xlT_ps = ps_tr.tile([128, 512], BF16, tag="tr")
            for k in range(4):
                nc.tensor.transpose(xlT_ps[:, 32 * k:32 * k + 32], xlnl[:, 128 * k:128 * k + 128], id32)
            xlT = trsb.tile([128, 128], BF16, tag="xlT")
            nc.vector.tensor_copy(xlT[:, :], xlT_ps[:, 0:128])

            # ============ Phase C: partial q_c / ckv, AllReduce ============
            qkvp = singles.tile([B, QL + D], BF16, tag="qkv", name="qkvp")
            wkva_t = wsm.tile([128, 4 * D], BF16, tag="wkva", name="wkva_t")
            nc.sync.dma_start(out=wkva_t[:, :].rearrange("p (k q) -> p k q", k=4),
                              in_=wkva.rearrange("(k p) q -> p k q", p=128))
            qa_ps = ps_ct4.tile([128, 512], F32, tag="ct4", name="qa_ps")
            for k in range(4):
                wqa_t = wbig.tile([128, 2048], BF16, tag="wbig", name="wqa_t")
                nc.sync.dma_start(out=wqa_t[:, 0:QL], in_=wqa[128 * k:128 * k + 128, :])
                for n in range(3):
                    nc.tensor.matmul(qa_ps[32 * n:32 * n + 32, :], xlT[:, 32 * k:32 * k + 32],
                                     wqa_t[:, 512 * n:512 * n + 512],
                                     start=(k == 0), stop=(k == 3),
                                     tile_position=(0, 32 * n))
            for n in range(3):
                nc.vector.tensor_copy(qkvp[:, 512 * n:512 * n + 512], qa_ps[32 * n:32 * n + 32, :])
            kv_ps = ps_ct4.tile([128, 512], F32, tag="ct4", name="kv_ps")
            for k in range(4):
                st, sp = (k == 0), (k == 3)
                nc.tensor.matmul(kv_ps[0:32, :], xlT[:, 32 * k:32 * k + 32],
                                 wkva_t[:, D * k:D * k + 512], start=st, stop=sp, tile_position=(0, 0))
                nc.tensor.matmul(kv_ps[32:64, 0:64], xlT[:, 32 * k:32 * k + 32],
                                 wkva_t[:, D * k + 512:D * k + 576], start=st, stop=sp, tile_position=(0, 32))
            nc.vector.tensor_copy(qkvp[:, QL:QL + 512], kv_ps[0:32, :])
            nc.vector.tensor_copy(qkvp[:, QL + 512:QL + 576], kv_ps[32:64, 0:64])

            nc.scalar.dma_start(out=c1i[:, :], in_=qkvp[:, :])
            if 'c1' in fake_coll:
                nc.sync.dma_start(out=c1o[:, :], in_=c1i[:, :])
            else:
                nc.gpsimd.collective_compute("AllReduce", mybir.AluOpType.add, replica_groups=rg,
                                             ins=[c1i.opt()], outs=[c1o.opt()])
            qkvf = singles.tile([B, QL + D], BF16, tag="qkv", name="qkvf")
            nc.scalar.dma_start(out=qkvf[:, :], in_=c1o[:, :])

            # ============ Phase D: norms, rope(k_pe), latent ============
            rstd_q = rms_rstd(qkvf[:, 0:QL], QL)
            lnq_b = bcast32(ln_q[:, :], QL, "lnq_b")
            qcn = singles.tile([B, QL], BF16)
            nc.vector.scalar_tensor_tensor(qcn[:, :], qkvf[:, 0:QL], rstd_q[:, :],
                                           lnq_b[:, :], mybir.AluOpType.mult, mybir.AluOpType.mult)

            rstd_kv = rms_rstd(qkvf[:, QL:QL + KVL], KVL)
            lnkv_b = bcast32(ln_kv[:, :], KVL, "lnkv_b")
            latent = singles.tile([B, D], BF16)
            nc.vector.scalar_tensor_tensor(latent[:, 0:KVL], qkvf[:, QL:QL + KVL], rstd_kv[:, :],
                                           lnkv_b[:, :], mybir.AluOpType.mult, mybir.AluOpType.mult)

            kpe_r = qkvf[:, QL + KVL:QL + D].rearrange("p (n two) -> p two n", two=2)
            rope(latent[:, KVL:KVL + 32], latent[:, KVL + 32:KVL + 64],
                 kpe_r[:, 0, :], kpe_r[:, 1, :])

            # local latent rows via selection matmul
            lsel_kv = ps_ct4.tile([4, 512], F32, tag="ct4", name="lsel_kv")
            nc.tensor.matmul(lsel_kv[:, :], bsel_t[:, :], latent[:, 0:KVL], start=True, stop=True)
            lsel_pe = ps_sc.tile([4, 64], F32, tag="sc", name="lsel_pe")
            nc.tensor.matmul(lsel_pe[:, :], bsel_t[:, :], latent[:, KVL:D], start=True, stop=True)
            lat_loc = singles.tile([B_LOC, D], BF16)
            nc.vector.tensor_copy(lat_loc[:, 0:KVL], lsel_kv[:, :])
            nc.vector.tensor_copy(lat_loc[:, KVL:D], lsel_pe[:, :])

            # latent rows flattened onto partition 0 (for rank-1 o_lat fold)
            lat_flat = singles.tile([1, B_LOC * D], BF16)
            for b in range(B_LOC):
                nc.sync.dma_start(out=lat_flat[0:1, D * b:D * b + D], in_=lat_loc[b:b + 1, :])

            # latT: [dlen, 4] per d-chunk dd at cols 4dd..4dd+4
            latT_ps = ps_tr.tile([128, 20], BF16, tag="tr", name="latT_ps")
            for dd in range(5):
                dlen = 128 if dd < 4 else 64
                nc.tensor.transpose(latT_ps[0:dlen, 4 * dd:4 * dd + 4],
                                    lat_loc[:, 128 * dd:128 * dd + dlen], ident[0:4, 0:4])
            latT = trsb.tile([128, 20], BF16, tag="latT")
            nc.vector.tensor_copy(latT[:, 0:16], latT_ps[:, 0:16])
            nc.vector.tensor_copy(latT[0:64, 16:20], latT_ps[0:64, 16:20])
            nc.vector.memset(latT[64:128, 16:20], 0.0)

            # ============ Phase E: local-head q, absorbed; AllToAll ============
            qcT_ps = ps_tr.tile([128, 512], BF16, tag="tr")
            for k in range(12):
                nc.tensor.transpose(qcT_ps[:, 32 * k:32 * k + 32],
                                    qcn[:, 128 * k:128 * k + 128], id32)
            qcT = trsb.tile([128, 384], BF16, tag="qcT")
            nc.vector.tensor_copy(qcT[:, :], qcT_ps[:, 0:384])

            qb_ps = ps_ct4.tile([128, 512], F32, tag="ct4", name="qb_ps")
            for k in range(12):
                st, sp = (k == 0), (k == 11)
                wqb_t = wbig.tile([128, 2048], BF16, tag="wbig", name="wqb_t")
                nc.sync.dma_start(out=wqb_t[:, 0:768], in_=wqb[128 * k:128 * k + 128, :])
                nc.tensor.matmul(qb_ps[0:32, :], qcT[:, 32 * k:32 * k + 32], wqb_t[:, 0:512],
                                 start=st, stop=sp, tile_position=(0, 0))
                nc.tensor.matmul(qb_ps[32:64, 0:256], qcT[:, 32 * k:32 * k + 32], wqb_t[:, 512:768],
                                 start=st, stop=sp, tile_position=(0, 32))
            q_loc = singles.tile([B, H_LOC * HEAD], BF16)
            nc.vector.tensor_copy(q_loc[:, 0:512], qb_ps[0:32, :])
            nc.vector.tensor_copy(q_loc[:, 512:768], qb_ps[32:64, 0:256])

            # q_nope^T for all 4 heads packed into one [128, 128] tile
            qnT_ps = ps_tr.tile([128, 128], BF16, tag="tr")
            for j in range(H_LOC):
                nc.tensor.transpose(qnT_ps[:, 32 * j:32 * j + 32],
                                    q_loc[:, HEAD * j:HEAD * j + NOPE], id32)
            qnT = trsb.tile([128, 128], BF16, tag="qnT")
            nc.vector.tensor_copy(qnT[:, :], qnT_ps[:, :])

            # W_UK^T direct load: ukT[n, 512j+l]
            ukT = wsm.tile([128, H_LOC * KVL], BF16, tag="ukt", name="ukT")
            nc.sync.dma_start(out=ukT[:, :], in_=ukt_i[:, :])

            qf = singles.tile([B, H_LOC * D], BF16)   # [32, 4h*576]
            qlp = ps_ct4.tile([128, 512], F32, tag="ct4", name="qlp")
            for j in range(H_LOC):
                qpe = q_loc[:, HEAD * j + NOPE:HEAD * (j + 1)].rearrange("p (n two) -> p two n", two=2)
                rope(qf[:, D * j + KVL:D * j + KVL + 32], qf[:, D * j + KVL + 32:D * j + D],
                     qpe[:, 0, :], qpe[:, 1, :])
                nc.tensor.matmul(qlp[32 * j:32 * j + 32, :],
                                 qnT[:, 32 * j:32 * j + 32],
                                 ukT[:, KVL * j:KVL * (j + 1)], start=True, stop=True,
                                 tile_position=(0, 32 * j))
            for j in range(H_LOC):
                nc.vector.tensor_copy(qf[:, D * j:D * j + KVL], qlp[32 * j:32 * j + 32, :])

            nc.scalar.dma_start(out=c2i[:, :], in_=qf[:, :])
            if 'c2' in fake_coll:
                nc.sync.dma_start(out=c2o[:, :], in_=c2i[:, :])
            else:
                nc.gpsimd.collective_compute("AllToAll", mybir.AluOpType.bypass, replica_groups=rg,
                                             ins=[c2i.opt()], outs=[c2o.opt()])

            # ============ Phase F: per-batch qT ============
            c2v = c2o[:, :].rearrange("(g b) (j d) -> g b j d", b=B_LOC, d=D)
            qT_b = []
            for b in range(B_LOC):
                qfb = mpool.tile([H, D], BF16, tag="qfb", bufs=2, name="qfb")
                nc.scalar.dma_start(
                    out=qfb[:, :],
                    in_=c2o[:, :].rearrange("(g b) (j d) -> b g j d", b=B_LOC, d=D)[b])
                qT_ps = ps_tr.tile([128, 160], BF16, tag="tr", name="qT_ps")
                for dd in range(5):
                    dlen = 128 if dd < 4 else 64
                    nc.tensor.transpose(qT_ps[0:dlen, 32 * dd:32 * dd + 32],
                                        qfb[:, 128 * dd:128 * dd + dlen], id32)
                qt = qtbp.tile([128, 160], BF16, tag="qtb", name="qt")
                nc.vector.tensor_copy(qt[:, 0:128], qT_ps[:, 0:128])
                nc.vector.tensor_copy(qt[0:64, 128:160], qT_ps[0:64, 128:160])
                nc.vector.memset(qt[64:128, 128:160], 0.0)
                qT_b.append(qt)

            # ============ Phase G: attention per local batch ============
            c3v = c3i[:, :].rearrange("(g b) (j l) -> g b j l", b=B_LOC, l=KVL)
            for b in range(B_LOC):
                denp = stats.tile([32, NST + 1], F32, tag="denp", bufs=4, name="denp")
                vps = ps_vps.tile([128, 512], F32, tag="vps", name="vps")
                grp_banks = None
                qt = qT_b[b]

                for st_i in range(NST):
                    ti = st_i % 4
                    if ti == 0:
                        grp_banks = [ps_ct4.tile([128, 512], F32, tag="ct4", name=f"scb{j}")
                                     for j in range(4)]
                    # cacheT tile for this s-tile: one contiguous [128, 2560] DMA
                    ctg = ctp.tile([128, 2560], BF16, tag="ctg", name="ctg")
                    nc.sync.dma_start(out=ctg[:, :], in_=cacheTp[b, st_i, :, :])

                    sc_ap = grp_banks[ti][32 * ti:32 * ti + 32, :]
                    tp = (0, 32 * ti)
                    for dd in range(5):
                        nc.tensor.matmul(sc_ap, qt[:, 32 * dd:32 * dd + 32],
                                         ctg[:, 512 * dd:512 * dd + 512],
                                         start=(dd == 0), stop=(dd == 4), tile_position=tp)

                    a_t = apool.tile([32, 512], BF16, tag="attn", name="a_t")
                    nc.scalar.activation(out=a_t[:, :], in_=sc_ap, func=AF.Exp,
                                         scale=SCALE, accum_out=denp[:, st_i:st_i + 1])

                    atT_ps = ps_tr.tile([128, 128], BF16, tag="tr", name="atT_ps")
                    for cc in range(4):
                        nc.tensor.transpose(atT_ps[:, 32 * cc:32 * cc + 32],
                                            a_t[:, 128 * cc:128 * cc + 128], id32)
                    atT = atp.tile([128, 128], BF16, tag="atT", name="atT")
                    nc.vector.tensor_copy(atT[:, :], atT_ps[:, :])

                    kvg = kvp.tile([128, 2048], BF16, tag="kvg", name="kvg")
                    nc.sync.dma_start(out=kvg[:, :], in_=cachekvp[b, st_i, :, :])
                    for cc in range(4):
                        nc.tensor.matmul(vps[32 * cc:32 * cc + 32, :], atT[:, 32 * cc:32 * cc + 32],
                                         kvg[:, 512 * cc:512 * cc + 512],
                                         start=(st_i == 0), stop=(st_i == 7),
                                         tile_position=(0, 32 * cc))

                # latent token: scores tail + exp
                tail_ps = ps_sc.tile([32, 512], F32, tag="sc", name="tail_ps")
                for dd in range(5):
                    nc.tensor.matmul(tail_ps[:, 0:1], qt[:, 32 * dd:32 * dd + 32],
                                     latT[0:128, 4 * dd + b:4 * dd + b + 1],
                                     start=(dd == 0), stop=(dd == 4))
                nc.scalar.activation(out=denp[:, NST:NST + 1], in_=tail_ps[:, 0:1],
                                     func=AF.Exp, scale=SCALE)
                wt_b = stats.tile([32, 1], BF16, tag="wtb", bufs=2, name="wt_b")
                nc.scalar.activation(out=wt_b[:, :], in_=tail_ps[:, 0:1],
                                     func=AF.Exp, scale=SCALE)
                wtT_ps = ps_tr.tile([1, 32], BF16, tag="tr", name="wtT_ps")
                nc.tensor.transpose(wtT_ps[:, :], wt_b[:, :], id32)
                wtT = stats.tile([1, 32], BF16, tag="wtT", bufs=2, name="wtT")
                nc.vector.tensor_copy(wtT[:, :], wtT_ps[:, :])

                vsb = mpool.tile([128, KVL], BF16, tag="vsb", bufs=2, name="vsb")
                nc.vector.tensor_copy(vsb[:, :], vps[:, :])
                olat_ps = ps_sc.tile([32, 512], F32, tag="sc", name="olat_ps")
                nc.tensor.matmul(olat_ps[:, :], sum_sel[:, :], vsb[:, :], start=True, stop=False)
                nc.tensor.matmul(olat_ps[:, :], wtT[:, :], lat_flat[0:1, D * b:D * b + KVL],
                                 start=False, stop=True)
                dsum = stats.tile([32, 1], F32, tag="st", name="dsum")
                nc.vector.reduce_sum(out=dsum[:, :], in_=denp[:, :], axis=AXX)
                drec = stats.tile([32, 1], F32, tag="st", name="drec")
                nc.vector.reciprocal(drec[:, :], dsum[:, :])
                ol_sb = mpool.tile([32, KVL], BF16, tag="olat_sb", bufs=2, name="ol_sb")
                nc.vector.tensor_scalar_mul(ol_sb[:, :], olat_ps[:, :], drec[:, :])
                nc.scalar.dma_start(
                    out=c3i[:, :].rearrange("(g b) (j l) -> b g j l", b=B_LOC, l=KVL)[b],
                    in_=ol_sb[:, :])

            if 'c3' in fake_coll:
                nc.sync.dma_start(out=c3o[:, :], in_=c3i[:, :])
            else:
                nc.gpsimd.collective_compute("AllToAll", mybir.AluOpType.bypass, replica_groups=rg,
                                             ins=[c3i.opt()], outs=[c3o.opt()])

            # ============ Phase H: W_UV + w_o, AllReduce -> h ============
            wuv_t = wsm.tile([128, 4 * H_LOC * V], BF16, tag="wuv", name="wuv_t")
            nc.sync.dma_start(out=wuv_t[:, :].rearrange("p (k v) -> p k v", k=4),
                              in_=wuv_i.rearrange("(k p) v -> p k v", p=128))
            UW = H_LOC * V  # 512
            c3ov = c3o[:, :].rearrange("(g b) (j l) -> g b j l", b=B_LOC, l=KVL)
            olT = []
            for j in range(H_LOC):
                olj = mpool.tile([B, KVL], BF16, tag="olj", bufs=2, name="olj")
                nc.scalar.dma_start(
                    out=olj[:, :],
                    in_=c3o[:, :].rearrange("(g b) (j l) -> j g b l", b=B_LOC, l=KVL)[j])
                olT_ps = ps_tr.tile([128, 128], BF16, tag="tr", name="olT_ps")
                for k in range(4):
                    nc.tensor.transpose(olT_ps[:, 32 * k:32 * k + 32],
                                        olj[:, 128 * k:128 * k + 128], id32)
                ot = trsb.tile([128, 128], BF16, tag="olT", bufs=4, name=f"olT{j}")
                nc.vector.tensor_copy(ot[:, :], olT_ps[:, :])
                olT.append(ot)

            oT_sb = trsb.tile([128, 128], BF16, tag="oT")
            oT_ps = ps_vps.tile([128, 512], F32, tag="vps", name="oT_ps")
            for j in range(H_LOC):
                for k in range(4):
                    nc.tensor.matmul(oT_ps[:, 32 * j:32 * j + 32],
                                     wuv_t[:, UW * k + 128 * j:UW * k + 128 * (j + 1)],
                                     olT[j][:, 32 * k:32 * k + 32], start=(k == 0), stop=(k == 3))
            nc.vector.tensor_copy(oT_sb[:, :], oT_ps[:, 0:128])

            hp = rows.tile([B, HID], BF16, tag="hpr", bufs=1, name="hp")
            wo_ps2 = [ps_ct4.tile([128, 512], F32, tag="ct4", name=f"wo_ps{wh}")
                      for wh in range(2)]
            for j in range(H_LOC):
                wo_t = wdp.tile([128, 4096], BF16, tag="wdp", name="wo_t")
                nc.sync.dma_start(out=wo_t[:, :], in_=wo[128 * j:128 * j + 128, :])
                for wh in range(2):
                    for n4 in range(4):
                        nc.tensor.matmul(wo_ps2[wh][32 * n4:32 * n4 + 32, :],
                                         oT_sb[:, 32 * j:32 * j + 32],
                                         wo_t[:, 2048 * wh + 512 * n4:2048 * wh + 512 * n4 + 512],
                                         start=(j == 0), stop=(j == 3),
                                         tile_position=(0, 32 * n4))
            for wh in range(2):
                for n4 in range(4):
                    n = 4 * wh + n4
                    nc.vector.tensor_copy(hp[:, 512 * n:512 * n + 512],
                                          wo_ps2[wh][32 * n4:32 * n4 + 32, :])

            nc.scalar.dma_start(out=c4i[:, :], in_=hp[:, :])
            if 'c4' in fake_coll:
                nc.sync.dma_start(out=c4o[:, :], in_=c4i[:, :])
            else:
                nc.gpsimd.collective_compute("AllReduce", mybir.AluOpType.add, replica_groups=rg,
                                             ins=[c4i.opt()], outs=[c4o.opt()])
            hsum = rows.tile([B, HID], BF16, tag="hpr", bufs=1, name="hsum")
            nc.scalar.dma_start(out=hsum[:, :], in_=c4o[:, :])
            hid2 = rows.tile([B, HID], F32, tag="rows", name="hid2")
            nc.sync.dma_start(out=hid2[:, :], in_=hidden[:, :])
            h_sb = singles.tile([B, HID], F32, tag="h_sb", name="h_sb")
            nc.vector.tensor_add(h_sb[:, :], hsum[:, :], hid2[:, :])
            nc.scalar.dma_start(out=h_out[:, :], in_=h_sb[:, :])

            # ============ Phase I: MLP ============
            rstd_p = rms_rstd(h_sb[:, :], HID)
            lnpost_b = bcast32(ln_post[:, :], HID, "lnpost_b")
            x2 = singles.tile([B, HID], BF16)
            nc.vector.scalar_tensor_tensor(x2[:, :], h_sb[:, :], rstd_p[:, :],
                                           lnpost_b[:, :], mybir.AluOpType.mult, mybir.AluOpType.mult)

            x2T = trsb.tile([128, 1024], BF16, tag="x2T")
            for half in range(2):
                x2T_ps = ps_tr.tile([128, 512], BF16, tag="tr", name="x2T_ps")
                for k in range(16):
                    kk = 16 * half + k
                    nc.tensor.transpose(x2T_ps[:, 32 * k:32 * k + 32],
                                        x2[:, 128 * kk:128 * kk + 128], id32)
                nc.vector.tensor_copy(x2T[:, 512 * half:512 * half + 512], x2T_ps[:, :])

            nwid = [512, 512, 352]
            gu_ps_all = [ps_ct4.tile([128, 512], F32, tag="ct4", name=f"gu_ps{hh}")
                         for hh in range(2)]
            for k in range(32):
                wg_t = wgup.tile([128, 2 * I_LOC], BF16, tag="wgup", name="wg_t")
                nc.sync.dma_start(out=wg_t[:, :], in_=wgu[128 * k:128 * k + 128, :])
                for half in range(2):
                    for n3 in range(3):
                        off = I_LOC * half + 512 * n3
                        nc.tensor.matmul(gu_ps_all[half][32 * n3:32 * n3 + 32, 0:nwid[n3]],
                                         x2T[:, 32 * k:32 * k + 32],
                                         wg_t[:, off:off + nwid[n3]], start=(k == 0), stop=(k == 31),
                                         tile_position=(0, 32 * n3))

            sil = rows.tile([32, I_LOC], F32, tag="sil", bufs=2, name="sil")
            act = singles.tile([32, I_LOC], BF16)
            for n3 in range(3):
                nc.scalar.activation(out=sil[:, 512 * n3:512 * n3 + nwid[n3]],
                                     in_=gu_ps_all[0][32 * n3:32 * n3 + 32, 0:nwid[n3]], func=AF.Silu)
                nc.vector.tensor_mul(act[:, 512 * n3:512 * n3 + nwid[n3]],
                                     sil[:, 512 * n3:512 * n3 + nwid[n3]],
                                     gu_ps_all[1][32 * n3:32 * n3 + 32, 0:nwid[n3]])

            actT = trsb.tile([128, 352], BF16, tag="actT")
            actT_ps = ps_tr.tile([128, 512], BF16, tag="tr", name="actT_ps")
            for k in range(11):
                klen = 128 if k < 10 else 96
                nc.tensor.transpose(actT_ps[0:klen, 32 * k:32 * k + 32],
                                    act[:, 128 * k:128 * k + klen], id32)
            nc.vector.tensor_copy(actT[:, 0:320], actT_ps[:, 0:320])
            nc.vector.tensor_copy(actT[0:96, 320:352], actT_ps[0:96, 320:352])

            mlp_sb = rows.tile([B, HID], F32, tag="rows", name="mlp_sb")
            dn_ps2 = [ps_ct4.tile([128, 512], F32, tag="ct4", name=f"dn_ps{bb}")
                      for bb in range(2)]
            for k in range(11):
                klen = 128 if k < 10 else 96
                wd_t = wdp.tile([128, 4096], BF16, tag="wdp", name="wd_t")
                nc.sync.dma_start(out=wd_t[0:klen, :],
                                  in_=wdn[128 * k:128 * k + klen, :])
                for bank in range(2):
                    for n in range(4):
                        nc.tensor.matmul(dn_ps2[bank][32 * n:32 * n + 32, :],
                                         actT[0:klen, 32 * k:32 * k + 32],
                                         wd_t[0:klen, 2048 * bank + 512 * n:2048 * bank + 512 * n + 512],
                                         start=(k == 0), stop=(k == 10),
                                         tile_position=(0, 32 * n))
            for bank in range(2):
                base = 2048 * bank
                for n in range(4):
                    nc.vector.tensor_copy(mlp_sb[:, base + 512 * n:base + 512 * n + 512],
                                          dn_ps2[bank][32 * n:32 * n + 32, :])
            nc.sync.dma_start(out=mlp_out[:, :], in_=mlp_sb[:, :])

    nc.compile()
    return nc


def _shard_inputs(inputs):
    bf = ml_dtypes.bfloat16
    hs = np.ascontiguousarray(inputs["hidden_states"], dtype=np.float32)
    cos = np.ascontiguousarray(inputs["cos"], dtype=np.float32)
    sin = np.ascontiguousarray(inputs["sin"], dtype=np.float32)
    kvc = np.asarray(inputs["kv_cache"], dtype=np.float32)
    w_q_a = np.asarray(inputs["w_q_a"]); w_q_b = np.asarray(inputs["w_q_b"])
    w_kv_a = np.asarray(inputs["w_kv_a"])
    w_kv_b = np.asarray(inputs["w_kv_b"]).reshape(KVL, H, NOPE + V)
    w_o = np.asarray(inputs["w_o"]); w_gu = np.asarray(inputs["w_gate_up"]); w_dn = np.asarray(inputs["w_down"])
    maps = []
    for i in range(NC_N):
        bsel = np.zeros((B, B_LOC), np.float32)
        for j in range(B_LOC):
            bsel[B_LOC * i + j, j] = 1.0
        kvc_l = kvc[B_LOC * i:B_LOC * (i + 1)]
        hsl = slice(H_LOC * i, H_LOC * (i + 1))
        uk = w_kv_b[:, hsl, :NOPE]                    # [KVL, 4, NOPE]
        # packed cacheT: [B_LOC, NST, 128, 5*512], d-tail (576->640) zero padded
        kvc_bf = kvc_l.astype(bf)
        ctp_h = np.zeros((B_LOC, NST, 128, 2560), bf)
        kvc_T = kvc_bf.transpose(0, 2, 1)             # [4, 576, 4096]
        for dd in range(5):
            dlen = 128 if dd < 4 else 64
            blk = kvc_T[:, 128 * dd:128 * dd + dlen, :].reshape(B_LOC, dlen, NST, 512)
            ctp_h[:, :, 0:dlen, 512 * dd:512 * dd + 512] = blk.transpose(0, 2, 1, 3)
        # packed cachekv: [B_LOC, NST, 128, 4*512]
        kvp_h = (kvc_bf[:, :, :KVL].reshape(B_LOC, NST, 4, 128, KVL)
                 .transpose(0, 1, 3, 2, 4).reshape(B_LOC, NST, 128, 2048))
        m = {
            "hidden": hs,
            "hidden_loc": np.ascontiguousarray(hs[:, HID_LOC * i:HID_LOC * (i + 1)]),
            "cos": cos, "sin": sin,
            "cacheTp": np.ascontiguousarray(ctp_h),
            "cachekvp": np.ascontiguousarray(kvp_h),
            "wqa": np.ascontiguousarray(w_q_a[HID_LOC * i:HID_LOC * (i + 1), :].astype(bf)),
            "wkva": np.ascontiguousarray(w_kv_a[HID_LOC * i:HID_LOC * (i + 1), :].astype(bf)),
            "wqb": np.ascontiguousarray(w_q_b[:, 768 * i:768 * (i + 1)].astype(bf)),
            "ukt": np.ascontiguousarray(uk.transpose(2, 1, 0).reshape(NOPE, H_LOC * KVL).astype(bf)),
            "wuv": np.ascontiguousarray(w_kv_b[:, hsl, NOPE:].reshape(KVL, H_LOC * V).astype(bf)),
            "wo": np.ascontiguousarray(w_o[512 * i:512 * (i + 1), :].astype(bf)),
            "wgu": np.ascontiguousarray(
                np.concatenate([w_gu[:, I_LOC * i:I_LOC * (i + 1)],
                                w_gu[:, INTER + I_LOC * i:INTER + I_LOC * (i + 1)]], axis=1).astype(bf)),
            "wdn": np.ascontiguousarray(w_dn[I_LOC * i:I_LOC * (i + 1), :].astype(bf)),
            "ln_in_loc": np.ascontiguousarray(np.tile(
                np.asarray(inputs["input_ln_w"][HID_LOC * i:HID_LOC * (i + 1)], dtype=np.float32)[None, :], (32, 1))),
            "ln_q": np.ascontiguousarray(np.tile(
                np.asarray(inputs["q_a_ln_w"], dtype=np.float32)[None, :], (32, 1))),
            "ln_kv": np.ascontiguousarray(np.tile(
                np.asarray(inputs["kv_a_ln_w"], dtype=np.float32)[None, :], (32, 1))),
            "ln_post": np.ascontiguousarray(np.tile(
                np.asarray(inputs["post_ln_w"], dtype=np.float32)[None, :], (32, 1))),
            "bsel": bsel.astype(bf),
        }
        maps.append(m)
    return maps


def kernel(**inputs):
    if "nc" not in _CACHE:
        _CACHE["nc"] = _build()
    nc = _CACHE["nc"]
    in_maps = _shard_inputs(inputs)
    res = run_bass_kernel_spmd(nc, in_maps, list(range(NC_N)))
    h = res.results[0]["h"]
    mlp = np.sum([res.results[i]["mlp"] for i in range(NC_N)], axis=0)
    return (h + mlp).astype(np.float32)


if __name__ == "__main__":
    data = np.load("/tmp/inputs.npz")
    inputs = {k: data[k] for k in data.files}
    out = kernel(**inputs)
    ref = np.load("/tmp/ref_out.npy")
    err = np.abs(out - ref).max()
    print("abs err:", err, "rel err:", err / np.abs(ref).max())
